# revision 1
# baseline (speedup 1.0000x reference)
"""Trainium2 Bass kernel for nn_AlternateLayer: stacked hidden-size-1 LSTMs.

Math (matching the jax reference):
  N = B*S = 2048 sequences. Per sequence: xf = flip(x, -1).reshape(T=30, 500).
  Layer 0: pre0[t] = xf[t] @ w_ih0.T + b_ih0 + b_hh0  (the only GEMM),
  then 64 stacked LSTM layers of hidden size 1 (layers 1..63 take the scalar
  h-stream of the layer below as input).

Implementation (v2):
  - Pure data parallelism: 256 sequences per NeuronCore (8 cores).
  - Wavefront over (layer l, time t): step s processes layers l with
    l + t = s, 93 steps. State H/C: [64 l, 256 n]; state writes extend DOWN to
    a 32-aligned partition start (rows below the active range belong to
    finished layers and are never read again, so garbage writes are safe).
  - Gate preacts come from TWO gate-pair matmuls per step into one PSUM tile
    G [128, 512]: cols 0:256 = pair (f: rows 0:64, i: rows 64:128),
    cols 256:512 = pair (o: rows 0:64, g~: rows 64:128). K is padded to 128
    (fast weight load). The augmented band matrix carries wih (subdiag),
    whh (diag), bias (via constant-1.0 H row 68), and pre0 delta rows -
    parity double-buffered at rows 64:68 (even t) / 96:100 (odd t) so the
    PSUM->SBUF pre0 copies stay off the critical chain.
  - ACT: one fused sigmoid over the (f,i) pair, sigmoid(o), tanh(g~),
    tanh(c). DVE does the c/h updates; all operands partition-start-aligned
    (TT output may be written at a shifted base if TT_OUT_SHIFT).
  - x is host-side reversed/transposed to [d=512(padded), t=30, n=256] bf16;
    recurrence state and band matmuls in bf16 (PSUM accumulation f32).
"""

import sys

sys.path.insert(0, "/opt/trn_rl_repo")

import numpy as np

import concourse.bacc as bacc
import concourse.bass as bass
import concourse.mybir as mybir
import concourse.tile as tile
from concourse.bass_utils import run_bass_kernel_spmd

B, S, T, D = 32, 64, 30, 500
L = 64
NCORES = 8
NPC = (B * S) // NCORES  # 256 sequences per core
DP = 512  # padded D
PERM = [0, 1, 3, 2]  # my gate order (i,f,o,g~) -> torch order (i,f,g,o)
NSTEPS = L + T - 1  # 93
KH = 128  # H rows: 64 state, 64:68 pre0-even, 68 ones, 96:100 pre0-odd
BF16 = mybir.dt.np(mybir.dt.bfloat16)
NSTREAM = 2  # independent batch streams per core (chain-latency hiding)

_CACHE = {}


def _build_program():
    nc = bacc.Bacc(
        "TRN2",
        target_bir_lowering=False,
        debug=False,
        enable_asserts=False,
        num_devices=NCORES,
    )
    f32 = mybir.dt.float32
    bf16 = mybir.dt.bfloat16
    ACT_SIG = mybir.ActivationFunctionType.Sigmoid
    ACT_TANH = mybir.ActivationFunctionType.Tanh
    ACT_COPY = mybir.ActivationFunctionType.Copy

    xt_d = nc.dram_tensor("xt", [DP, T * NPC], bf16, kind="ExternalInput").ap()
    wg_d = nc.dram_tensor("wg", [4, 128, KH], bf16, kind="ExternalInput").ap()
    # wband[k, pair, par, m]: k contraction row, m = pair-gate column
    wband_d = nc.dram_tensor(
        "wband", [KH, 2, 2, KH], bf16, kind="ExternalInput"
    ).ap()
    ones_d = nc.dram_tensor("onesrow", [1, NPC], bf16, kind="ExternalInput").ap()
    out_d = nc.dram_tensor("out", [T, NPC], bf16, kind="ExternalOutput").ap()

    with tile.TileContext(nc) as tc:
        import contextlib

        with contextlib.ExitStack() as ctx:
            consts = ctx.enter_context(tc.tile_pool(name="consts", bufs=1))
            xpool = ctx.enter_context(tc.tile_pool(name="x", bufs=1))
            state = ctx.enter_context(tc.tile_pool(name="state", bufs=1))
            ypool = ctx.enter_context(tc.tile_pool(name="y", bufs=3))
            tpool = ctx.enter_context(tc.tile_pool(name="tmp", bufs=3))
            gpool = ctx.enter_context(tc.tile_pool(name="g", bufs=2, space="PSUM"))
            ppool = ctx.enter_context(tc.tile_pool(name="p", bufs=2, space="PSUM"))

            wband = consts.tile([KH, 2, 2, KH], bf16)
            nc.sync.dma_start(wband[:], wband_d[:])
            wg = []
            for c in range(4):
                w = consts.tile([128, KH], bf16, tag=f"wg{c}", name=f"wg{c}")
                nc.sync.dma_start(w[:], wg_d[c])
                wg.append(w)

            NTG = 6
            TG = T // NTG  # 5 timesteps per DMA group
            xt = []
            for c in range(4):
                xt.append(
                    xpool.tile([128, T * NPC], bf16, tag=f"xt{c}", name=f"xt{c}")
                )
            for tg in range(NTG):
                cs, ce = tg * TG * NPC, (tg + 1) * TG * NPC
                for c in range(4):
                    nc.sync.dma_start(
                        xt[c][:, cs:ce], xt_d[c * 128 : (c + 1) * 128, cs:ce]
                    )

            NW = NPC // NSTREAM  # free width per stream
            Hs, Cs = [], []
            for q in range(NSTREAM):
                Hq = state.tile([KH, NW], bf16, tag=f"H{q}", name=f"H{q}")
                Cq = state.tile([L, NW], bf16, tag=f"C{q}", name=f"C{q}")
                nc.vector.memset(Hq[:], 0.0)
                nc.vector.memset(Cq[:], 0.0)
                nc.sync.dma_start(Hq[68:69, :], ones_d[:, 0:NW])
                Hs.append(Hq)
                Cs.append(Cq)

            # layer-0 GEMM groups (2 timesteps each); ppool throttles
            NPAIR = T // 2
            pre0 = []
            for p in range(NPAIR):
                P = ppool.tile([KH, 2 * NPC], f32, tag="P", name="P")
                for c in range(4):
                    nc.tensor.matmul(
                        P[:],
                        wg[c][:],
                        xt[c][:, (2 * p) * NPC : (2 * p + 2) * NPC],
                        start=(c == 0),
                        stop=(c == 3),
                    )
                pre0.append(P)

            # --- wavefront ---
            for s in range(NSTEPS):
                lo = max(0, s - (T - 1))
                hi = min(L - 1, s)
                par = s % 2
                # even steps contract over rows [0:69) only — this excludes
                # the odd-parity pre0 rows (96:100) so odd copies can land
                # ahead of time without a WAR against these matmuls.
                kk = 69 if par == 0 else KH
                r0 = 64 if par == 0 else 96
                a0 = 32 * (lo // 32)
                sl = slice(a0, hi + 1)  # base-0 aligned active superset

                for q in range(NSTREAM):
                    H, C = Hs[q], Cs[q]
                    if s <= T - 1:
                        # pre0 rows for t = s (parity picks the row group)
                        P = pre0[s // 2]
                        c0 = par * NPC + q * NW
                        nc.scalar.activation(
                            H[r0 : r0 + 4, :],
                            P[r0 : r0 + 4, c0 : c0 + NW],
                            ACT_COPY,
                        )

                    G = gpool.tile([KH, 2 * NW], f32, tag=f"G{q}", name=f"G{q}")
                    for pair in range(2):
                        nc.tensor.matmul(
                            G[:, pair * NW : (pair + 1) * NW],
                            wband[0:kk, pair, par, :],
                            H[0:kk],
                            start=True,
                            stop=True,
                        )

                    # All-tanh gates with state scaling D=2c, H=2h:
                    # pair 0 = (f rows 0:64, i rows 64:128),
                    # pair 1 = (g~ rows 0:64, o rows 64:128); preacts for
                    # f,i,o are pre-scaled x0.5 in the band matrix so
                    # sigma(x) = (tanh(x/2)+1)/2 folds into the updates:
                    #   u = (Yf+1)*D = 4*sig(f)*c,  v = (Yi+1)*Yg = 2*sig(i)*g
                    #   D' = 0.5*u + v = 2c',  TC = tanh(0.5*D') = tanh(c')
                    #   H' = (Yo+1)*TC = 2h'
                    Ylo = ypool.tile(
                        [L, 2 * NW], bf16, tag=f"Ylo{q}", name=f"Ylo{q}"
                    )
                    Yhi = ypool.tile(
                        [L, 2 * NW], bf16, tag=f"Yhi{q}", name=f"Yhi{q}"
                    )
                    nc.scalar.activation(Ylo[:], G[0:64, :], ACT_TANH)
                    nc.scalar.activation(Yhi[:], G[64:128, :], ACT_TANH)

                    MUL = mybir.AluOpType.mult
                    ADD = mybir.AluOpType.add
                    u = tpool.tile([L, NW], bf16, tag=f"u{q}", name=f"u{q}")
                    v = tpool.tile([L, NW], bf16, tag=f"v{q}", name=f"v{q}")
                    tc_ = tpool.tile([L, NW], bf16, tag=f"tc{q}", name=f"tc{q}")
                    # u = (Yf + 1) * D
                    nc.vector.scalar_tensor_tensor(
                        u[sl], Ylo[sl, 0:NW], 1.0, C[sl], op0=ADD, op1=MUL
                    )
                    # v = (Yi + 1) * Yg
                    nc.vector.scalar_tensor_tensor(
                        v[sl], Yhi[sl, 0:NW], 1.0, Ylo[sl, NW : 2 * NW],
                        op0=ADD, op1=MUL,
                    )
                    # D' = 0.5*u + v
                    nc.vector.scalar_tensor_tensor(
                        C[sl], u[sl], 0.5, v[sl], op0=MUL, op1=ADD
                    )
                    # TC = tanh(0.5 * D')
                    nc.scalar.activation(tc_[sl], C[sl], ACT_TANH, scale=0.5)
                    # H' = (Yo + 1) * TC
                    nc.vector.scalar_tensor_tensor(
                        H[sl], Yhi[sl, NW : 2 * NW], 1.0, tc_[sl],
                        op0=ADD, op1=MUL,
                    )
                    if s >= L - 1:
                        t = s - (L - 1)
                        nc.sync.dma_start(
                            out_d[t : t + 1, q * NW : (q + 1) * NW],
                            H[L - 1 : L, :],
                        )

    nc.compile()
    return nc


def _prep_core_inputs(x_shard, w_ih0, w_hh0, b_ih0, b_hh0, w_ih, w_hh, b_ih, b_hh):
    """Host-side prep of one core's input arrays."""
    xr = x_shard[:, ::-1].astype(np.float32)  # [NPC, 15000]
    xr = np.ascontiguousarray(xr).reshape(NPC, T, D)
    xp = np.zeros((NPC, T, DP), dtype=np.float32)
    xp[:, :, :D] = xr
    xt = np.ascontiguousarray(xp.transpose(2, 1, 0).reshape(DP, T * NPC))
    xt = xt.astype(BF16)

    # wg: [4][128, KH]; cols 64+g (even) and 96+g (odd) carry gate g weights
    wpad = np.zeros((DP, 4), dtype=np.float32)
    for g in range(4):
        wpad[:D, g] = w_ih0[PERM[g], :]
    wg = np.zeros((4, 128, KH), dtype=np.float32)
    for c in range(4):
        for g in range(4):
            wg[c, :, 64 + g] = wpad[c * 128 : (c + 1) * 128, g]
            wg[c, :, 96 + g] = wpad[c * 128 : (c + 1) * 128, g]
    wg = wg.astype(BF16)

    # wband[k, pair, par, m]: pair0 = (f: m 0:64, i: m 64:128),
    # pair1 = (g~: m 0:64, o: m 64:128). My gate ids: 0=i, 1=f, 2=o, 3=g~.
    # Scale folding for the all-tanh / doubled-state scheme: state rows hold
    # 2h so wih/whh scale x0.5; f,i,o preacts additionally scale x0.5 so
    # sigma comes out of tanh. All factors are powers of two (exact in bf16).
    MYGATE = {(0, 0): 1, (0, 1): 0, (1, 0): 3, (1, 1): 2}  # (pair, half) -> g
    wband = np.zeros((KH, 2, 2, KH), dtype=np.float32)
    for pair in range(2):
        for half in range(2):
            g = MYGATE[(pair, half)]
            tg = PERM[g]
            argsc = 0.5 if g != 3 else 1.0  # sigma-arg halving (not g~)
            hsc = 0.5 * argsc  # state rows: h = H/2
            for par in range(2):
                col = 64 * half
                wband[0, pair, par, col + 0] = hsc * w_hh0[tg, 0]
                for l in range(1, L):
                    wband[l - 1, pair, par, col + l] = hsc * w_ih[l - 1, tg, 0]
                    wband[l, pair, par, col + l] = hsc * w_hh[l - 1, tg, 0]
                wband[68, pair, par, col + 0] = argsc * (b_ih0[tg] + b_hh0[tg])
                wband[68, pair, par, col + 1 : col + L] = argsc * (
                    b_ih[:, tg] + b_hh[:, tg]
                )
                # pre0 delta row for layer 0 (parity selects the row group)
                r0 = 64 if par == 0 else 96
                wband[r0 + g, pair, par, col + 0] = argsc
    return {
        "xt": xt,
        "wg": wg,
        "wband": wband.astype(BF16),
        "onesrow": np.ones((1, NPC), dtype=BF16),
    }


def _run(inputs, trace=False, trace_kwargs=None):
    if "nc" not in _CACHE:
        _CACHE["nc"] = _build_program()
    nc = _CACHE["nc"]

    x = np.asarray(inputs["x"], dtype=np.float32).reshape(B * S, T * D)
    params = {
        k: np.asarray(inputs[k], dtype=np.float32)
        for k in ("w_ih0", "w_hh0", "b_ih0", "b_hh0", "w_ih", "w_hh", "b_ih", "b_hh")
    }
    in_maps = []
    for i in range(NCORES):
        shard = x[i * NPC : (i + 1) * NPC]
        in_maps.append(_prep_core_inputs(shard, **params))

    res = run_bass_kernel_spmd(
        nc,
        in_maps,
        core_ids=list(range(NCORES)),
        trace=trace,
        **(trace_kwargs or {}),
    )

    out = np.empty((B * S, T), dtype=np.float32)
    for i in range(NCORES):
        # device stores H = 2h (doubled state); halve on the host
        out[i * NPC : (i + 1) * NPC] = (
            np.asarray(res.results[i]["out"]).astype(np.float32).T * 0.5
        )
    return out.reshape(B, S, T), res


def kernel(**inputs):
    out, _ = _run(inputs, trace=False)
    return out



# revision 12
# speedup vs baseline: 1.0343x; 1.0343x over previous
"""Trainium2 Bass kernel for nn_AlternateLayer: stacked hidden-size-1 LSTMs.

Math (matching the jax reference):
  N = B*S = 2048 sequences. Per sequence: xf = flip(x, -1).reshape(T=30, 500).
  Layer 0: pre0[t] = xf[t] @ w_ih0.T + b_ih0 + b_hh0  (the only GEMM),
  then 64 stacked LSTM layers of hidden size 1 (layers 1..63 take the scalar
  h-stream of the layer below as input).

Implementation (v2):
  - Pure data parallelism: 256 sequences per NeuronCore (8 cores).
  - Wavefront over (layer l, time t): step s processes layers l with
    l + t = s, 93 steps. State H/C: [64 l, 256 n]; state writes extend DOWN to
    a 32-aligned partition start (rows below the active range belong to
    finished layers and are never read again, so garbage writes are safe).
  - Gate preacts come from TWO gate-pair matmuls per step into one PSUM tile
    G [128, 512]: cols 0:256 = pair (f: rows 0:64, i: rows 64:128),
    cols 256:512 = pair (o: rows 0:64, g~: rows 64:128). K is padded to 128
    (fast weight load). The augmented band matrix carries wih (subdiag),
    whh (diag), bias (via constant-1.0 H row 68), and pre0 delta rows -
    parity double-buffered at rows 64:68 (even t) / 96:100 (odd t) so the
    PSUM->SBUF pre0 copies stay off the critical chain.
  - ACT: one fused sigmoid over the (f,i) pair, sigmoid(o), tanh(g~),
    tanh(c). DVE does the c/h updates; all operands partition-start-aligned
    (TT output may be written at a shifted base if TT_OUT_SHIFT).
  - x is host-side reversed/transposed to [d=512(padded), t=30, n=256] bf16;
    recurrence state and band matmuls in bf16 (PSUM accumulation f32).
"""

import sys

sys.path.insert(0, "/opt/trn_rl_repo")

import numpy as np

import concourse.bacc as bacc
import concourse.bass as bass
import concourse.mybir as mybir
import concourse.tile as tile
from concourse.bass_utils import run_bass_kernel_spmd

B, S, T, D = 32, 64, 30, 500
L = 64
NCORES = 8
NPC = (B * S) // NCORES  # 256 sequences per core
DP = 512  # padded D
PERM = [0, 1, 3, 2]  # my gate order (i,f,o,g~) -> torch order (i,f,g,o)
NSTEPS = L + T - 1  # 93
KH = 128  # H rows: 64 state, 64:68 pre0-even, 68 ones, 96:100 pre0-odd
BF16 = mybir.dt.np(mybir.dt.bfloat16)
NSTREAM = 2  # independent batch streams per core (chain-latency hiding)

_CACHE = {}


def _build_program():
    nc = bacc.Bacc(
        "TRN2",
        target_bir_lowering=False,
        debug=False,
        enable_asserts=False,
        num_devices=NCORES,
    )
    f32 = mybir.dt.float32
    bf16 = mybir.dt.bfloat16
    ACT_SIG = mybir.ActivationFunctionType.Sigmoid
    ACT_TANH = mybir.ActivationFunctionType.Tanh
    ACT_COPY = mybir.ActivationFunctionType.Copy

    xt_d = nc.dram_tensor("xt", [DP, T * NPC], bf16, kind="ExternalInput").ap()
    wg_d = nc.dram_tensor("wg", [4, 128, KH], bf16, kind="ExternalInput").ap()
    # wband[k, pair, par, m]: k contraction row, m = pair-gate column
    wband_d = nc.dram_tensor(
        "wband", [KH, 2, 2, KH], bf16, kind="ExternalInput"
    ).ap()
    ones_d = nc.dram_tensor("onesrow", [1, NPC], bf16, kind="ExternalInput").ap()
    out_d = nc.dram_tensor("out", [1, T * NPC], bf16, kind="ExternalOutput").ap()

    with tile.TileContext(nc) as tc:
        import contextlib

        with contextlib.ExitStack() as ctx:
            consts = ctx.enter_context(tc.tile_pool(name="consts", bufs=1))
            xpool = ctx.enter_context(tc.tile_pool(name="x", bufs=1))
            state = ctx.enter_context(tc.tile_pool(name="state", bufs=1))
            ypool = ctx.enter_context(tc.tile_pool(name="y", bufs=2))
            tpool = ctx.enter_context(tc.tile_pool(name="tmp", bufs=2))
            gpool = ctx.enter_context(tc.tile_pool(name="g", bufs=2, space="PSUM"))
            ppool = ctx.enter_context(tc.tile_pool(name="p", bufs=2, space="PSUM"))

            wband = consts.tile([KH, 2, 2, KH], bf16)
            nc.sync.dma_start(wband[:], wband_d[:])
            wg = []
            for c in range(4):
                w = consts.tile([128, KH], bf16, tag=f"wg{c}", name=f"wg{c}")
                nc.sync.dma_start(w[:], wg_d[c])
                wg.append(w)

            NTG = 6
            TG = T // NTG  # 5 timesteps per DMA group
            xt = []
            for c in range(4):
                xt.append(
                    xpool.tile([128, T * NPC], bf16, tag=f"xt{c}", name=f"xt{c}")
                )
            for tg in range(NTG):
                cs, ce = tg * TG * NPC, (tg + 1) * TG * NPC
                for c in range(4):
                    nc.sync.dma_start(
                        xt[c][:, cs:ce], xt_d[c * 128 : (c + 1) * 128, cs:ce]
                    )

            NW = NPC // NSTREAM  # free width per stream
            Hs, Cs = [], []
            for q in range(NSTREAM):
                Hq = state.tile([KH, NW], bf16, tag=f"H{q}", name=f"H{q}")
                Cq = state.tile([L, NW], bf16, tag=f"C{q}", name=f"C{q}")
                nc.vector.memset(Hq[:], 0.0)
                nc.vector.memset(Cq[:], 0.0)
                nc.sync.dma_start(Hq[68:69, :], ones_d[:, 0:NW])
                Hs.append(Hq)
                Cs.append(Cq)
            # output staging: per step a 32-aligned block copy of H[32:64]
            # lands at cols t*NPC..; row 31 (= layer 63) is DMA'd at the end
            OUT = state.tile([32, T * NPC], bf16, tag="OUT", name="OUT")

            # layer-0 GEMM groups (2 timesteps each); ppool throttles
            NPAIR = T // 2
            pre0 = []
            for p in range(NPAIR):
                P = ppool.tile([KH, 2 * NPC], f32, tag="P", name="P")
                for c in range(4):
                    nc.tensor.matmul(
                        P[:],
                        wg[c][:],
                        xt[c][:, (2 * p) * NPC : (2 * p + 2) * NPC],
                        start=(c == 0),
                        stop=(c == 3),
                    )
                pre0.append(P)

            MUL = mybir.AluOpType.mult
            ADD = mybir.AluOpType.add

            def pre0_copy(s):
                # pre0 rows for step s (parity picks the row group). Issued
                # one step early (during step s-1, after that step's gate
                # ACT): legal because the only prior reader of these rows is
                # the step-s-2 matmul (even matmuls contract [0:69) and skip
                # the odd rows; even rows are read by every matmul, so the
                # window is exactly (mm(s-1), mm(s)]).
                par_ = s % 2
                r0 = 64 if par_ == 0 else 96
                P = pre0[s // 2]
                for q in range(NSTREAM):
                    c0 = par_ * NPC + q * NW
                    nc.scalar.activation(
                        Hs[q][r0 : r0 + 4, :],
                        P[r0 : r0 + 4, c0 : c0 + NW],
                        ACT_COPY,
                    )

            # --- wavefront ---
            # state scaling D=2c, H=2h (all-tanh gates):
            # pair 0 = (f rows 0:64, i rows 64:128),
            # pair 1 = (o rows 0:64, g~ rows 64:128); preacts for f,i,o are
            # pre-scaled x0.5 in the band matrix so
            # sigma(x) = (tanh(x/2)+1)/2 folds into the updates:
            #   u = (Yf+1)*D = 4*sig(f)*c,  v = (Yi+1)*Yg = 2*sig(i)*g
            #   D' = 0.5*u + v = 2c',  TC = tanh(0.5*D') = tanh(c')
            #   H' = (Yo+1)*TC = 2h'
            pre0_copy(0)
            pre0_copy(1)
            for s in range(NSTEPS):
                lo = max(0, s - (T - 1))
                hi = min(L - 1, s)
                par = s % 2
                # even steps contract over rows [0:69) only — this excludes
                # the odd-parity pre0 rows (96:100) so copies can land a
                # step ahead without a WAR against these matmuls.
                kk = 69 if par == 0 else KH
                a0 = 32 * (lo // 32)
                sl = slice(a0, hi + 1)  # base-0 aligned active superset
                shi = slice(64 + a0, 64 + hi + 1)

                Gs, Ys = [], []
                for q in range(NSTREAM):
                    G = gpool.tile([KH, 2 * NW], f32, tag=f"G{q}", name=f"G{q}")
                    for pair in range(2):
                        nc.tensor.matmul(
                            G[:, pair * NW : (pair + 1) * NW],
                            wband[0:kk, pair, par, :],
                            Hs[q][0:kk],
                            start=True,
                            stop=True,
                        )
                    Gs.append(G)
                # one fused tanh over all 128 gate rows per stream
                for q in range(NSTREAM):
                    Y = ypool.tile(
                        [KH, 2 * NW], bf16, tag=f"Y{q}", name=f"Y{q}"
                    )
                    nc.scalar.activation(Y[:], Gs[q][:], ACT_TANH)
                    Ys.append(Y)
                if s + 2 <= T - 1:
                    pre0_copy(s + 2)
                us, vs = [], []
                for q in range(NSTREAM):
                    Y = Ys[q]
                    u = tpool.tile([L, NW], bf16, tag=f"u{q}", name=f"u{q}")
                    v = tpool.tile([L, NW], bf16, tag=f"v{q}", name=f"v{q}")
                    # u = (Yf + 1) * D
                    nc.vector.scalar_tensor_tensor(
                        u[sl], Y[sl, 0:NW], 1.0, Cs[q][sl], op0=ADD, op1=MUL
                    )
                    # v = (Yi + 1) * Yg  (both inputs in the bottom half)
                    nc.vector.scalar_tensor_tensor(
                        v[sl], Y[shi, 0:NW], 1.0, Y[shi, NW : 2 * NW],
                        op0=ADD, op1=MUL,
                    )
                    us.append(u)
                    vs.append(v)
                for q in range(NSTREAM):
                    # D' = 0.5*u + v
                    nc.vector.scalar_tensor_tensor(
                        Cs[q][sl], us[q][sl], 0.5, vs[q][sl], op0=MUL, op1=ADD
                    )
                tcs = []
                for q in range(NSTREAM):
                    tc_ = tpool.tile([L, NW], bf16, tag=f"tc{q}", name=f"tc{q}")
                    # TC = tanh(0.5 * D')
                    nc.scalar.activation(tc_[sl], Cs[q][sl], ACT_TANH, scale=0.5)
                    tcs.append(tc_)
                for q in range(NSTREAM):
                    # H' = (Yo + 1) * TC  (o is in the top half)
                    nc.vector.scalar_tensor_tensor(
                        Hs[q][sl], Ys[q][sl, NW : 2 * NW], 1.0, tcs[q][sl],
                        op0=ADD, op1=MUL,
                    )
                if s >= L - 1:
                    t = s - (L - 1)
                    for q in range(NSTREAM):
                        c0 = t * NPC + q * NW
                        nc.vector.tensor_copy(
                            OUT[0:32, c0 : c0 + NW], Hs[q][32:64, :]
                        )
            nc.sync.dma_start(out_d[:], OUT[31:32, :])

    nc.compile()
    return nc


def _prep_core_inputs(x_shard, w_ih0, w_hh0, b_ih0, b_hh0, w_ih, w_hh, b_ih, b_hh):
    """Host-side prep of one core's input arrays."""
    xr = x_shard[:, ::-1].astype(np.float32)  # [NPC, 15000]
    xr = np.ascontiguousarray(xr).reshape(NPC, T, D)
    xp = np.zeros((NPC, T, DP), dtype=np.float32)
    xp[:, :, :D] = xr
    xt = np.ascontiguousarray(xp.transpose(2, 1, 0).reshape(DP, T * NPC))
    xt = xt.astype(BF16)

    # wg: [4][128, KH]; cols 64+g (even) and 96+g (odd) carry gate g weights
    wpad = np.zeros((DP, 4), dtype=np.float32)
    for g in range(4):
        wpad[:D, g] = w_ih0[PERM[g], :]
    wg = np.zeros((4, 128, KH), dtype=np.float32)
    for c in range(4):
        for g in range(4):
            wg[c, :, 64 + g] = wpad[c * 128 : (c + 1) * 128, g]
            wg[c, :, 96 + g] = wpad[c * 128 : (c + 1) * 128, g]
    wg = wg.astype(BF16)

    # wband[k, pair, par, m]: pair0 = (f: m 0:64, i: m 64:128),
    # pair1 = (o: m 0:64, g~: m 64:128). My gate ids: 0=i, 1=f, 2=o, 3=g~.
    # (i and g~ share the bottom half so the v-STT's two inputs have equal
    # base partitions — a walrus requirement for SBUF/SBUF TensorScalarPtr.)
    # Scale folding for the all-tanh / doubled-state scheme: state rows hold
    # 2h so wih/whh scale x0.5; f,i,o preacts additionally scale x0.5 so
    # sigma comes out of tanh. All factors are powers of two (exact in bf16).
    MYGATE = {(0, 0): 1, (0, 1): 0, (1, 0): 2, (1, 1): 3}  # (pair, half) -> g
    wband = np.zeros((KH, 2, 2, KH), dtype=np.float32)
    for pair in range(2):
        for half in range(2):
            g = MYGATE[(pair, half)]
            tg = PERM[g]
            argsc = 0.5 if g != 3 else 1.0  # sigma-arg halving (not g~)
            hsc = 0.5 * argsc  # state rows: h = H/2
            for par in range(2):
                col = 64 * half
                wband[0, pair, par, col + 0] = hsc * w_hh0[tg, 0]
                for l in range(1, L):
                    wband[l - 1, pair, par, col + l] = hsc * w_ih[l - 1, tg, 0]
                    wband[l, pair, par, col + l] = hsc * w_hh[l - 1, tg, 0]
                wband[68, pair, par, col + 0] = argsc * (b_ih0[tg] + b_hh0[tg])
                wband[68, pair, par, col + 1 : col + L] = argsc * (
                    b_ih[:, tg] + b_hh[:, tg]
                )
                # pre0 delta row for layer 0 (parity selects the row group)
                r0 = 64 if par == 0 else 96
                wband[r0 + g, pair, par, col + 0] = argsc
    return {
        "xt": xt,
        "wg": wg,
        "wband": wband.astype(BF16),
        "onesrow": np.ones((1, NPC), dtype=BF16),
    }


def _run(inputs, trace=False, trace_kwargs=None):
    if "nc" not in _CACHE:
        _CACHE["nc"] = _build_program()
    nc = _CACHE["nc"]

    x = np.asarray(inputs["x"], dtype=np.float32).reshape(B * S, T * D)
    params = {
        k: np.asarray(inputs[k], dtype=np.float32)
        for k in ("w_ih0", "w_hh0", "b_ih0", "b_hh0", "w_ih", "w_hh", "b_ih", "b_hh")
    }
    in_maps = []
    for i in range(NCORES):
        shard = x[i * NPC : (i + 1) * NPC]
        in_maps.append(_prep_core_inputs(shard, **params))

    res = run_bass_kernel_spmd(
        nc,
        in_maps,
        core_ids=list(range(NCORES)),
        trace=trace,
        **(trace_kwargs or {}),
    )

    out = np.empty((B * S, T), dtype=np.float32)
    for i in range(NCORES):
        # device stores H = 2h (doubled state); halve on the host
        o = np.asarray(res.results[i]["out"]).astype(np.float32).reshape(T, NPC)
        out[i * NPC : (i + 1) * NPC] = o.T * 0.5
    return out.reshape(B, S, T), res


def kernel(**inputs):
    out, _ = _run(inputs, trace=False)
    return out



# revision 15
# speedup vs baseline: 1.1887x; 1.1493x over previous
"""Trainium2 Bass kernel for nn_AlternateLayer: stacked hidden-size-1 LSTMs.

Math (matching the jax reference):
  N = B*S = 2048 sequences. Per sequence: xf = flip(x, -1).reshape(T=30, 500).
  Layer 0: pre0[t] = xf[t] @ w_ih0.T + b_ih0 + b_hh0  (the only GEMM),
  then 64 stacked LSTM layers of hidden size 1 (layers 1..63 take the scalar
  h-stream of the layer below as input).

Implementation (v2):
  - Pure data parallelism: 256 sequences per NeuronCore (8 cores).
  - Wavefront over (layer l, time t): step s processes layers l with
    l + t = s, 93 steps. State H/C: [64 l, 256 n]; state writes extend DOWN to
    a 32-aligned partition start (rows below the active range belong to
    finished layers and are never read again, so garbage writes are safe).
  - Gate preacts come from TWO gate-pair matmuls per step into one PSUM tile
    G [128, 512]: cols 0:256 = pair (f: rows 0:64, i: rows 64:128),
    cols 256:512 = pair (o: rows 0:64, g~: rows 64:128). K is padded to 128
    (fast weight load). The augmented band matrix carries wih (subdiag),
    whh (diag), bias (via constant-1.0 H row 68), and pre0 delta rows -
    parity double-buffered at rows 64:68 (even t) / 96:100 (odd t) so the
    PSUM->SBUF pre0 copies stay off the critical chain.
  - ACT: one fused sigmoid over the (f,i) pair, sigmoid(o), tanh(g~),
    tanh(c). DVE does the c/h updates; all operands partition-start-aligned
    (TT output may be written at a shifted base if TT_OUT_SHIFT).
  - x is host-side reversed/transposed to [d=512(padded), t=30, n=256] bf16;
    recurrence state and band matmuls in bf16 (PSUM accumulation f32).
"""

import sys

sys.path.insert(0, "/opt/trn_rl_repo")

import numpy as np

import concourse.bacc as bacc
import concourse.bass as bass
import concourse.mybir as mybir
import concourse.tile as tile
from concourse.bass_utils import run_bass_kernel_spmd

B, S, T, D = 32, 64, 30, 500
L = 64
NCORES = 8
NPC = (B * S) // NCORES  # 256 sequences per core
DP = 512  # padded D
PERM = [0, 1, 3, 2]  # my gate order (i,f,o,g~) -> torch order (i,f,g,o)
NSTEPS = L + T - 1  # 93
KH = 128  # H rows: 64 state, 64:68 pre0-even, 68 ones, 96:100 pre0-odd
BF16 = mybir.dt.np(mybir.dt.bfloat16)
NSTREAM = 2  # independent batch streams per core (chain-latency hiding)

_CACHE = {}


def _build_program():
    nc = bacc.Bacc(
        "TRN2",
        target_bir_lowering=False,
        debug=False,
        enable_asserts=False,
        num_devices=NCORES,
    )
    f32 = mybir.dt.float32
    bf16 = mybir.dt.bfloat16
    ACT_SIG = mybir.ActivationFunctionType.Sigmoid
    ACT_TANH = mybir.ActivationFunctionType.Tanh
    ACT_COPY = mybir.ActivationFunctionType.Copy

    xt_d = nc.dram_tensor("xt", [DP, T * NPC], bf16, kind="ExternalInput").ap()
    wg_d = nc.dram_tensor("wg", [4, 128, KH], bf16, kind="ExternalInput").ap()
    # wband[k, pair, par, m]: k contraction row, m = pair-gate column
    wband_d = nc.dram_tensor(
        "wband", [KH, 2, 2, KH], bf16, kind="ExternalInput"
    ).ap()
    ones_d = nc.dram_tensor("onesrow", [1, NPC], bf16, kind="ExternalInput").ap()
    out_d = nc.dram_tensor("out", [1, T * NPC], bf16, kind="ExternalOutput").ap()

    with tile.TileContext(nc) as tc:
        import contextlib

        with contextlib.ExitStack() as ctx:
            consts = ctx.enter_context(tc.tile_pool(name="consts", bufs=1))
            xpool = ctx.enter_context(tc.tile_pool(name="x", bufs=1))
            state = ctx.enter_context(tc.tile_pool(name="state", bufs=1))
            ypool = ctx.enter_context(tc.tile_pool(name="y", bufs=2))
            tpool = ctx.enter_context(tc.tile_pool(name="tmp", bufs=2))
            gpool = ctx.enter_context(tc.tile_pool(name="g", bufs=2, space="PSUM"))
            ppool = ctx.enter_context(tc.tile_pool(name="p", bufs=2, space="PSUM"))
            dpool = ctx.enter_context(tc.tile_pool(name="d", bufs=1, space="PSUM"))

            wband = consts.tile([KH, 2, 2, KH], bf16)
            nc.sync.dma_start(wband[:], wband_d[:])
            wg = []
            for c in range(4):
                w = consts.tile([128, KH], bf16, tag=f"wg{c}", name=f"wg{c}")
                nc.sync.dma_start(w[:], wg_d[c])
                wg.append(w)

            NTG = 6
            TG = T // NTG  # 5 timesteps per DMA group
            xt = []
            for c in range(4):
                xt.append(
                    xpool.tile([128, T * NPC], bf16, tag=f"xt{c}", name=f"xt{c}")
                )
            for tg in range(NTG):
                cs, ce = tg * TG * NPC, (tg + 1) * TG * NPC
                for c in range(4):
                    nc.sync.dma_start(
                        xt[c][:, cs:ce], xt_d[c * 128 : (c + 1) * 128, cs:ce]
                    )

            NW = NPC // NSTREAM  # free width per stream
            Hs, Cs = [], []
            for q in range(NSTREAM):
                Hq = state.tile([KH, NW], bf16, tag=f"H{q}", name=f"H{q}")
                Cq = state.tile([L, NW], bf16, tag=f"C{q}", name=f"C{q}")
                nc.vector.memset(Hq[:], 0.0)
                nc.vector.memset(Cq[:], 0.0)
                nc.sync.dma_start(Hq[68:69, :], ones_d[:, 0:NW])
                Hs.append(Hq)
                Cs.append(Cq)
            # output staging: per step a 32-aligned block copy of H[32:64]
            # lands at cols t*NPC..; row 31 (= layer 63) is DMA'd at the end
            OUT = state.tile([32, T * NPC], bf16, tag="OUT", name="OUT")

            # layer-0 GEMM groups (2 timesteps each); ppool throttles
            NPAIR = T // 2
            pre0 = []
            for p in range(NPAIR):
                P = ppool.tile([KH, 2 * NPC], f32, tag="P", name="P")
                for c in range(4):
                    nc.tensor.matmul(
                        P[:],
                        wg[c][:],
                        xt[c][:, (2 * p) * NPC : (2 * p + 2) * NPC],
                        start=(c == 0),
                        stop=(c == 3),
                    )
                pre0.append(P)

            MUL = mybir.AluOpType.mult
            ADD = mybir.AluOpType.add
            DUM = dpool.tile([KH, 512], f32, tag="DUM", name="DUM")

            def pre0_copy(s):
                # pre0 rows for step s (parity picks the row group). Issued
                # one step early (during step s-1, after that step's gate
                # ACT): legal because the only prior reader of these rows is
                # the step-s-2 matmul (even matmuls contract [0:69) and skip
                # the odd rows; even rows are read by every matmul, so the
                # window is exactly (mm(s-1), mm(s)]).
                par_ = s % 2
                r0 = 64 if par_ == 0 else 96
                P = pre0[s // 2]
                for q in range(NSTREAM):
                    c0 = par_ * NPC + q * NW
                    nc.scalar.activation(
                        Hs[q][r0 : r0 + 4, :],
                        P[r0 : r0 + 4, c0 : c0 + NW],
                        ACT_COPY,
                    )

            # --- wavefront ---
            # state scaling D=2c, H=2h (all-tanh gates):
            # pair 0 = (f rows 0:64, i rows 64:128),
            # pair 1 = (o rows 0:64, g~ rows 64:128); preacts for f,i,o are
            # pre-scaled x0.5 in the band matrix so
            # sigma(x) = (tanh(x/2)+1)/2 folds into the updates:
            #   u = (Yf+1)*D = 4*sig(f)*c,  v = (Yi+1)*Yg = 2*sig(i)*g
            #   D' = 0.5*u + v = 2c',  TC = tanh(0.5*D') = tanh(c')
            #   H' = (Yo+1)*TC = 2h'
            pre0_copy(0)
            pre0_copy(1)
            for s in range(NSTEPS):
                lo = max(0, s - (T - 1))
                hi = min(L - 1, s)
                par = s % 2
                # even steps contract over rows [0:69) only — this excludes
                # the odd-parity pre0 rows (96:100) so copies can land a
                # step ahead without a WAR against these matmuls.
                kk = 69 if par == 0 else KH
                a0 = 32 * (lo // 32)
                sl = slice(a0, hi + 1)  # base-0 aligned active superset
                shi = slice(64 + a0, 64 + hi + 1)

                Gs, Y0s, Y1s = [], [], []
                for q in range(NSTREAM):
                    G = gpool.tile([KH, 2 * NW], f32, tag=f"G{q}", name=f"G{q}")
                    for pair in range(2):
                        nc.tensor.matmul(
                            G[:, pair * NW : (pair + 1) * NW],
                            wband[0:kk, pair, par, :],
                            Hs[q][0:kk],
                            start=True,
                            stop=True,
                        )
                    Gs.append(G)
                if s >= 26:
                    # keep the PE's HAM clock-gate warm (K=8/8) once the
                    # layer-0 GEMM stream dries up: two throwaway matmuls
                    # into a scratch bank each step
                    for d in range(2):
                        nc.tensor.matmul(
                            DUM[:], wband[0:128, 0, 0, :], xt[0][:, 0:512],
                            start=True, stop=True,
                        )
                # pair0 = (f,i) via sigmoid(2x); pair1 = (o,g~) via tanh
                for q in range(NSTREAM):
                    Y0 = ypool.tile([KH, NW], bf16, tag=f"Y0{q}", name=f"Y0{q}")
                    Y1 = ypool.tile([KH, NW], bf16, tag=f"Y1{q}", name=f"Y1{q}")
                    nc.scalar.activation(Y0[:], Gs[q][:, 0:NW], ACT_SIG, scale=2.0)
                    nc.scalar.activation(Y1[:], Gs[q][:, NW : 2 * NW], ACT_TANH)
                    Y0s.append(Y0)
                    Y1s.append(Y1)
                if s + 2 <= T - 1:
                    pre0_copy(s + 2)
                tcs = []
                for q in range(NSTREAM):
                    u = tpool.tile([L, NW], bf16, tag=f"u{q}", name=f"u{q}")
                    v = tpool.tile([L, NW], bf16, tag=f"v{q}", name=f"v{q}")
                    # u = sig(f) * c
                    nc.vector.tensor_mul(u[sl], Y0s[q][sl], Cs[q][sl])
                    # v = sig(i) * tanh(g)  (both in the bottom half)
                    nc.vector.tensor_mul(v[sl], Y0s[q][shi], Y1s[q][shi])
                    # c' = u + v
                    nc.vector.tensor_add(Cs[q][sl], u[sl], v[sl])
                for q in range(NSTREAM):
                    tc_ = tpool.tile([L, NW], bf16, tag=f"tc{q}", name=f"tc{q}")
                    # TC = tanh(c')
                    nc.scalar.activation(tc_[sl], Cs[q][sl], ACT_TANH)
                    tcs.append(tc_)
                for q in range(NSTREAM):
                    # H' = (Yo + 1) * TC = 2h'  (o is pair1's top half)
                    nc.vector.scalar_tensor_tensor(
                        Hs[q][sl], Y1s[q][sl], 1.0, tcs[q][sl],
                        op0=ADD, op1=MUL,
                    )
                if s >= L - 1:
                    t = s - (L - 1)
                    for q in range(NSTREAM):
                        c0 = t * NPC + q * NW
                        nc.vector.tensor_copy(
                            OUT[0:32, c0 : c0 + NW], Hs[q][32:64, :]
                        )
            nc.sync.dma_start(out_d[:], OUT[31:32, :])

    nc.compile()
    return nc


def _prep_core_inputs(x_shard, w_ih0, w_hh0, b_ih0, b_hh0, w_ih, w_hh, b_ih, b_hh):
    """Host-side prep of one core's input arrays."""
    xr = x_shard[:, ::-1].astype(np.float32)  # [NPC, 15000]
    xr = np.ascontiguousarray(xr).reshape(NPC, T, D)
    xp = np.zeros((NPC, T, DP), dtype=np.float32)
    xp[:, :, :D] = xr
    xt = np.ascontiguousarray(xp.transpose(2, 1, 0).reshape(DP, T * NPC))
    xt = xt.astype(BF16)

    # wg: [4][128, KH]; cols 64+g (even) and 96+g (odd) carry gate g weights
    wpad = np.zeros((DP, 4), dtype=np.float32)
    for g in range(4):
        wpad[:D, g] = w_ih0[PERM[g], :]
    wg = np.zeros((4, 128, KH), dtype=np.float32)
    for c in range(4):
        for g in range(4):
            wg[c, :, 64 + g] = wpad[c * 128 : (c + 1) * 128, g]
            wg[c, :, 96 + g] = wpad[c * 128 : (c + 1) * 128, g]
    wg = wg.astype(BF16)

    # wband[k, pair, par, m]: pair0 = (f: m 0:64, i: m 64:128),
    # pair1 = (o: m 0:64, g~: m 64:128). My gate ids: 0=i, 1=f, 2=o, 3=g~.
    # (i and g~ share the bottom half so the v-STT's two inputs have equal
    # base partitions — a walrus requirement for SBUF/SBUF TensorScalarPtr.)
    # Scale folding for the all-tanh / doubled-state scheme: state rows hold
    # 2h so wih/whh scale x0.5; f,i,o preacts additionally scale x0.5 so
    # sigma comes out of tanh. All factors are powers of two (exact in bf16).
    MYGATE = {(0, 0): 1, (0, 1): 0, (1, 0): 2, (1, 1): 3}  # (pair, half) -> g
    wband = np.zeros((KH, 2, 2, KH), dtype=np.float32)
    for pair in range(2):
        for half in range(2):
            g = MYGATE[(pair, half)]
            tg = PERM[g]
            argsc = 0.5 if g != 3 else 1.0  # sigma-arg halving (not g~)
            hsc = 0.5 * argsc  # state rows: h = H/2
            for par in range(2):
                col = 64 * half
                wband[0, pair, par, col + 0] = hsc * w_hh0[tg, 0]
                for l in range(1, L):
                    wband[l - 1, pair, par, col + l] = hsc * w_ih[l - 1, tg, 0]
                    wband[l, pair, par, col + l] = hsc * w_hh[l - 1, tg, 0]
                wband[68, pair, par, col + 0] = argsc * (b_ih0[tg] + b_hh0[tg])
                wband[68, pair, par, col + 1 : col + L] = argsc * (
                    b_ih[:, tg] + b_hh[:, tg]
                )
                # pre0 delta row for layer 0 (parity selects the row group)
                r0 = 64 if par == 0 else 96
                wband[r0 + g, pair, par, col + 0] = argsc
    return {
        "xt": xt,
        "wg": wg,
        "wband": wband.astype(BF16),
        "onesrow": np.ones((1, NPC), dtype=BF16),
    }


def _run(inputs, trace=False, trace_kwargs=None):
    if "nc" not in _CACHE:
        _CACHE["nc"] = _build_program()
    nc = _CACHE["nc"]

    x = np.asarray(inputs["x"], dtype=np.float32).reshape(B * S, T * D)
    params = {
        k: np.asarray(inputs[k], dtype=np.float32)
        for k in ("w_ih0", "w_hh0", "b_ih0", "b_hh0", "w_ih", "w_hh", "b_ih", "b_hh")
    }
    in_maps = []
    for i in range(NCORES):
        shard = x[i * NPC : (i + 1) * NPC]
        in_maps.append(_prep_core_inputs(shard, **params))

    res = run_bass_kernel_spmd(
        nc,
        in_maps,
        core_ids=list(range(NCORES)),
        trace=trace,
        **(trace_kwargs or {}),
    )

    out = np.empty((B * S, T), dtype=np.float32)
    for i in range(NCORES):
        # device stores H = 2h (doubled state); halve on the host
        o = np.asarray(res.results[i]["out"]).astype(np.float32).reshape(T, NPC)
        out[i * NPC : (i + 1) * NPC] = o.T * 0.5
    return out.reshape(B, S, T), res


def kernel(**inputs):
    out, _ = _run(inputs, trace=False)
    return out

